# revision 1
# baseline (speedup 1.0000x reference)
"""Trainium2 Bass kernel for nn_DecoderLayer (decode attention + FFN).

Data-parallel over batch B=8 across 8 NeuronCores; weights replicated.
Per core: K/V projections of [4096,1024] @ [1024,1024] (the dominant
compute), single-query attention over S=4096, then the per-token decode
epilogue (Wo projection, LayerNorm, FFN, LayerNorm).

Matmuls run in float32r (single-pass fp32 PE mode, ~12-13 mantissa bits,
full 1 cycle/row rate); accumulation is fp32 in PSUM. Activations are
transposed on the PE (exact fp32).
"""
import sys

sys.path.insert(0, "/opt/trn_rl_repo")

import numpy as np

import concourse.bass as bass
import concourse.tile as tile
from concourse import bacc, mybir
from concourse.masks import make_identity

F32 = mybir.dt.float32
F32R = mybir.dt.float32r

N_CORES = 8
S = 4096          # kv sequence length per core (one batch)
D = 1024          # model dim
H = 16            # heads
DH = 64           # head dim
F = 4096          # ffn hidden
P = 128           # partitions
NK = D // P       # 8 contraction chunks over D
SBLK = 512        # s-block width for pass 1/2
NSB = S // SBLK   # 8
EPS = 1e-6
SCALE = 1.0 / 32.0  # 1/sqrt(D)

_CACHE = {}


def _build(reps=1):
    nc = bacc.Bacc("TRN2", target_bir_lowering=False, debug=False,
                   num_devices=N_CORES)

    dk = nc.dram_tensor("key", [S, D], F32, kind="ExternalInput").ap()
    dv = nc.dram_tensor("value", [S, D], F32, kind="ExternalInput").ap()
    ddec = nc.dram_tensor("dec", [1, D], F32, kind="ExternalInput").ap()
    dWq = nc.dram_tensor("Wq", [D, D], F32, kind="ExternalInput").ap()
    dWk = nc.dram_tensor("Wk", [D, D], F32, kind="ExternalInput").ap()
    dWv = nc.dram_tensor("Wv", [D, D], F32, kind="ExternalInput").ap()
    dWo = nc.dram_tensor("Wo", [D, D], F32, kind="ExternalInput").ap()
    dW1s = nc.dram_tensor("W1s", [D, 512], F32, kind="ExternalInput").ap()
    dW2s = nc.dram_tensor("W2s", [512, D], F32, kind="ExternalInput").ap()
    dbq = nc.dram_tensor("bq", [1, D], F32, kind="ExternalInput").ap()
    dbv = nc.dram_tensor("bv", [1, D], F32, kind="ExternalInput").ap()
    dbo = nc.dram_tensor("bo", [1, D], F32, kind="ExternalInput").ap()
    db1s = nc.dram_tensor("b1s", [1, 512], F32, kind="ExternalInput").ap()
    db2 = nc.dram_tensor("b2", [1, D], F32, kind="ExternalInput").ap()
    dg2 = nc.dram_tensor("ln2_g", [1, D], F32, kind="ExternalInput").ap()
    dl2 = nc.dram_tensor("ln2_b", [1, D], F32, kind="ExternalInput").ap()
    dgf = nc.dram_tensor("lnf_g", [1, D], F32, kind="ExternalInput").ap()
    dlf = nc.dram_tensor("lnf_b", [1, D], F32, kind="ExternalInput").ap()
    dout = nc.dram_tensor("out", [1, D], F32, kind="ExternalOutput").ap()

    env = locals()
    with tile.TileContext(nc) as tc:
        for _ in range(reps):
            _emit(nc, tc, env)
    nc.compile()
    return nc


def _emit(nc, tc, t):
    from contextlib import ExitStack
    ctx = ExitStack()
    with ctx:
        persist = ctx.enter_context(tc.tile_pool(name="persist", bufs=1))
        dram = ctx.enter_context(tc.tile_pool(name="dram", bufs=1, space="DRAM"))
        nat = ctx.enter_context(tc.tile_pool(name="nat", bufs=3))
        bigw = ctx.enter_context(tc.tile_pool(name="bigw", bufs=1))
        actT = ctx.enter_context(tc.tile_pool(name="actT", bufs=1))
        mmout = ctx.enter_context(tc.tile_pool(name="mmout", bufs=3))
        stream = ctx.enter_context(tc.tile_pool(name="stream", bufs=3))
        small = ctx.enter_context(tc.tile_pool(name="small", bufs=1))
        ps_tr = ctx.enter_context(tc.tile_pool(name="ps_tr", bufs=2, space="PSUM"))
        ps_mm = ctx.enter_context(tc.tile_pool(name="ps_mm", bufs=2, space="PSUM"))
        ps_sm = ctx.enter_context(tc.tile_pool(name="ps_sm", bufs=2, space="PSUM"))
        ps_ctx = ctx.enter_context(tc.tile_pool(name="ps_ctx", bufs=1, space="PSUM"))

        # ---------------- setup ----------------
        ident = persist.tile([P, P], F32)
        make_identity(nc, ident)

        # persistent fp32r weights: Wv (persist), Wk (bigw slot 1)
        Wv_r = persist.tile([P, NK, D], F32R)
        nc.gpsimd.dma_start(out=Wv_r, in_=t["dWv"].rearrange("(n p) d -> p n d", p=P))
        Wk_r = bigw.tile([P, NK, D], F32R, tag="bigw")
        nc.gpsimd.dma_start(out=Wk_r, in_=t["dWk"].rearrange("(n p) d -> p n d", p=P))


        def wstream_rhs(view, tag):
            def f(c, nb):
                w_t = stream.tile([P, 512], F32, tag="wstage",
                                  name=f"w{tag}_{c}_{nb}")
                nc.sync.dma_start(out=w_t,
                                  in_=view[:, c, nb * 512:(nb + 1) * 512])
                w_r = stream.tile([P, 512], F32R, tag="wstream",
                                  name=f"wr{tag}_{c}_{nb}")
                nc.vector.tensor_copy(out=w_r, in_=w_t)
                return w_r
            return f

        Wo_view = t["dWo"].rearrange("(n p) d -> p n d", p=P)
        Wq_view = t["dWq"].rearrange("(n p) d -> p n d", p=P)


        dec_sb = persist.tile([1, D], F32)
        nc.sync.dma_start(out=dec_sb, in_=t["ddec"])
        bq_sb = persist.tile([1, D], F32)
        nc.sync.dma_start(out=bq_sb, in_=t["dbq"])
        bo_sb = persist.tile([1, D], F32)
        nc.sync.dma_start(out=bo_sb, in_=t["dbo"])
        b2_sb = persist.tile([1, D], F32)
        nc.sync.dma_start(out=b2_sb, in_=t["db2"])
        b1s_bc = persist.tile([8, 512], F32)
        nc.sync.dma_start(out=b1s_bc, in_=bass.AP(
            tensor=t["db1s"].tensor, offset=t["db1s"].offset,
            ap=[[0, 8], [1, 512]]))
        bv_row = persist.tile([1, D], F32)
        nc.sync.dma_start(out=bv_row, in_=t["dbv"])
        g2_sb = persist.tile([1, D], F32)
        nc.sync.dma_start(out=g2_sb, in_=t["dg2"])
        l2_sb = persist.tile([1, D], F32)
        nc.sync.dma_start(out=l2_sb, in_=t["dl2"])
        gf_sb = persist.tile([1, D], F32)
        nc.sync.dma_start(out=gf_sb, in_=t["dgf"])
        lf_sb = persist.tile([1, D], F32)
        nc.sync.dma_start(out=lf_sb, in_=t["dlf"])
        eps_sb = persist.tile([1, 1], F32)
        nc.vector.memset(eps_sb, EPS)

        # head indicator Ehead[p, c, h] = 1 iff h == 2c + p//64
        Ehead = persist.tile([P, NK, H], F32)
        nc.gpsimd.memset(Ehead, 0.0)
        for c in range(NK):
            nc.gpsimd.memset(Ehead[0:64, c, 2 * c:2 * c + 1], 1.0)
            nc.gpsimd.memset(Ehead[64:P, c, 2 * c + 1:2 * c + 2], 1.0)

        # helpers ------------------------------------------------------
        def matvec_to_cols(xT_r, rhs_of, nk, nout, bias_row, out_dtype, name,
                           relu=False):
            """x @ W + b -> column layout [128, nout//128] tile (dtype out_dtype).

            Each 512-wide output block: psum accumulate -> +bias -> (relu) ->
            transient row -> 4 PE transposes -> copy into the column tile.
            """
            colT = small.tile([P, nout // P], out_dtype, tag="colT", bufs=2,
                              name=f"colT_{name}")
            for nb in range(nout // 512):
                pv = ps_sm.tile([1, 512], F32, tag="bankA", name=f"pv_{name}{nb}")
                for c in range(nk):
                    nc.tensor.matmul(pv, xT_r[:, c:c + 1], rhs_of(c, nb),
                                     start=(c == 0), stop=(c == nk - 1))
                row = small.tile([1, 512], F32, tag="row512", bufs=2,
                                 name=f"row_{name}{nb}")
                if bias_row is not None:
                    nc.vector.tensor_add(out=row, in0=pv,
                                         in1=bias_row(nb))
                else:
                    nc.vector.tensor_copy(row, pv)
                if relu:
                    nc.scalar.activation(row, row,
                                         mybir.ActivationFunctionType.Relu)
                pT = ps_sm.tile([P, 4], F32, tag="bankA", name=f"pT_{name}{nb}")
                for c in range(4):
                    nc.tensor.transpose(pT[:, c:c + 1], row[:, c * P:(c + 1) * P],
                                        ident[0:1, 0:1])
                nc.vector.tensor_copy(colT[:, nb * 4:(nb + 1) * 4], pT)
            return colT

        def row_transpose(row_sb, nchunk, name):
            """[1, nchunk*128] sbuf row -> [128, nchunk] f32r sbuf tile."""
            pT = ps_sm.tile([P, nchunk], F32, tag="bankA", name=f"pTr_{name}")
            for c in range(nchunk):
                nc.tensor.transpose(pT[:, c:c + 1],
                                    row_sb[:, c * P:(c + 1) * P],
                                    ident[0:1, 0:1])
            colT = small.tile([P, nchunk], F32R, tag="colT", bufs=2,
                              name=f"colTr_{name}")
            nc.vector.tensor_copy(colT, pT)
            return colT

        def bias_slices(row_tile):
            return lambda nb: row_tile[:, nb * 512:(nb + 1) * 512]

        # ---------------- prologue: Q2 ----------------
        decT_r = row_transpose(dec_sb, NK, "dec")
        v1T_r = matvec_to_cols(
            decT_r, lambda c, nb: Wv_r[:, c, nb * 512:(nb + 1) * 512],
            NK, D, bias_slices(bv_row), F32R, "v1")
        m1T_r = matvec_to_cols(v1T_r, wstream_rhs(Wo_view, "oa"),
                               NK, D, bias_slices(bo_sb), F32R, "m1")
        qT_sb = matvec_to_cols(m1T_r, wstream_rhs(Wq_view, "q"),
                               NK, D, bias_slices(bq_sb), F32, "q2")
        qexp_r = persist.tile([P, NK, H], F32R)
        for c in range(NK):
            nc.vector.tensor_scalar_mul(out=qexp_r[:, c, :], in0=Ehead[:, c, :],
                                        scalar1=qT_sb[:, c:c + 1])

        # ---------------- pass 1: K^T projection + scores ----------------
        scores_sb = persist.tile([H, S], F32)

        def load_transposed(src_dram, sb, name):
            """One 512-row s-block -> actT tile [P, NK, SBLK] f32r (= X^T)."""
            xT = actT.tile([P, NK, SBLK], F32R, tag="actT", name=f"xT_{name}{sb}")
            for j in range(SBLK // P):
                xn = nat.tile([P, D], F32, tag="nat", name=f"nat_{name}{sb}_{j}")
                nc.sync.dma_start(
                    out=xn, in_=src_dram[sb * SBLK + j * P: sb * SBLK + (j + 1) * P, :])
                for g in range(2):
                    ptr = ps_tr.tile([P, 4, P], F32, tag="ptr")
                    for q in range(4):
                        dc = g * 4 + q
                        nc.tensor.transpose(ptr[:, q, :],
                                            xn[:, dc * P:(dc + 1) * P], ident)
                    if g == 0:
                        nc.vector.tensor_copy(
                            xT[:, g * 4:(g + 1) * 4, j * P:(j + 1) * P], ptr)
                    else:
                        nc.scalar.copy(
                            out=xT[:, g * 4:(g + 1) * 4, j * P:(j + 1) * P],
                            in_=ptr)
            return xT

        for sb in range(NSB):
            keyT = load_transposed(t["dk"], sb, "k")
            psc = ps_sm.tile([H, SBLK], F32, tag="bankA", name=f"psc{sb}")
            for dout in range(NK):
                pKT = ps_mm.tile([P, SBLK], F32, tag="pmm", name=f"pKT{sb}_{dout}")
                for kc in range(NK):
                    nc.tensor.matmul(pKT, Wk_r[:, kc, dout * P:(dout + 1) * P],
                                     keyT[:, kc, :],
                                     start=(kc == 0), stop=(kc == NK - 1))
                KT_r = mmout.tile([P, SBLK], F32R, tag="mmout", name=f"KT{sb}_{dout}")
                nc.vector.tensor_copy(KT_r, pKT)
                nc.tensor.matmul(psc, qexp_r[:, dout, :], KT_r,
                                 start=(dout == 0), stop=(dout == NK - 1))
            nc.vector.tensor_copy(scores_sb[:, sb * SBLK:(sb + 1) * SBLK], psc)

        # FFN weight slices reuse the Wk slot after pass 1:
        # Wffn[:, :, 0:512] = W1s^T-chunks, Wffn[:, :, 512:1024] = W2s chunks
        Wffn = bigw.tile([P, NK, D], F32R, tag="bigw")
        nc.gpsimd.dma_start(out=Wffn[:, :, 0:512],
                            in_=t["dW1s"].rearrange("(n p) f -> p n f", p=P))
        W2s_in = t["dW2s"].rearrange("(n p) d -> p n d", p=P)
        Wffn_v = Wffn.rearrange("p (c e) d -> p c e d", e=2)
        for nh in range(2):
            nc.gpsimd.dma_start(
                out=Wffn_v[:, :, nh, 512:D],
                in_=W2s_in[:, :, nh * 512:(nh + 1) * 512])

        # ---------------- softmax (scale folded into exp) ----------------
        mx = small.tile([H, 1], F32)
        nc.vector.reduce_max(mx, scores_sb, axis=mybir.AxisListType.X)
        nmx = small.tile([H, 1], F32)
        nc.scalar.mul(nmx, mx, -SCALE)
        nc.scalar.activation(scores_sb, scores_sb,
                             mybir.ActivationFunctionType.Exp,
                             bias=nmx, scale=SCALE)
        zsum = small.tile([H, 1], F32)
        nc.vector.reduce_sum(zsum, scores_sb, axis=mybir.AxisListType.X)
        rz = small.tile([H, 1], F32)
        nc.vector.reciprocal(rz, zsum)

        # w^T chunks [s=128, h=16] for ctx matmuls
        wT_r = persist.tile([P, S // P, H], F32R)
        for g in range(S // P // 8):
            pwT = ps_tr.tile([P, 8, H], F32, tag="ptr", name=f"pwT{g}")
            for q in range(8):
                ch = g * 8 + q
                nc.tensor.transpose(pwT[:, q, :],
                                    scores_sb[:, ch * P:(ch + 1) * P],
                                    ident[0:H, 0:H])
            nc.vector.tensor_copy(wT_r[:, g * 8:(g + 1) * 8, :], pwT)

        # ---------------- pass 2: V projection + ctx ----------------
        pctx = ps_ctx.tile([H, D], F32, tag="ctx")
        for sb in range(NSB):
            valT = load_transposed(t["dv"], sb, "v")
            for j in range(SBLK // P):
                ch = sb * (SBLK // P) + j
                V_r = mmout.tile([P, 2, 512], F32R, tag="mmout", name=f"V{ch}")
                for nh in range(2):
                    pV = ps_mm.tile([P, 512], F32, tag="pmm", name=f"pV{ch}_{nh}")
                    for kc in range(NK):
                        nc.tensor.matmul(pV, valT[:, kc, j * P:(j + 1) * P],
                                         Wv_r[:, kc, nh * 512:(nh + 1) * 512],
                                         start=(kc == 0), stop=(kc == NK - 1))
                    nc.vector.tensor_copy(V_r[:, nh, :], pV)
                for nh in range(2):
                    nc.tensor.matmul(pctx[:, nh * 512:(nh + 1) * 512],
                                     wT_r[:, ch, :], V_r[:, nh, :],
                                     start=(ch == 0), stop=(ch == S // P - 1))

        # normalize ctx rows by 1/Z_h while copying out of PSUM
        ctx_sb = small.tile([H, D], F32, tag="rowtmp", bufs=2)
        nc.vector.tensor_scalar_mul(out=ctx_sb, in0=pctx, scalar1=rz)

        # ---------------- epilogue ----------------
        # diagonal head extraction via DRAM bounce:
        # ctx_diag[c*128+p] = ctx[h, 64h+j], h=2c+p//64, j=p%64
        scratch = dram.tile([H, D], F32)
        nc.sync.dma_start(out=scratch, in_=ctx_sb)
        fl = scratch.flatten()
        diag_in = bass.AP(tensor=fl.tensor, offset=fl.offset,
                          ap=[[1088, H], [1, DH]])
        ctxd_row = small.tile([1, D], F32, tag="rowtmp", bufs=2)
        nc.sync.dma_start(out=ctxd_row.rearrange("o (g j) -> o g j", j=DH),
                          in_=diag_in)
        nc.vector.tensor_add(out=ctxd_row, in0=ctxd_row, in1=bv_row)
        ctxdT_r = row_transpose(ctxd_row, NK, "ctxd")

        # mha2 = ctx_diag @ Wo + bo ; u = mha2 + dec ; x = LN(u)
        u_sb = small.tile([1, D], F32)
        for nb in range(2):
            pv = ps_sm.tile([1, 512], F32, tag="bankA", name=f"pm2_{nb}")
            for c in range(NK):
                nc.tensor.matmul(pv, ctxdT_r[:, c:c + 1],
                                 wstream_rhs(Wo_view, "ob")(c, nb),
                                 start=(c == 0), stop=(c == NK - 1))
            sl = slice(nb * 512, (nb + 1) * 512)
            nc.vector.tensor_add(out=u_sb[:, sl], in0=pv, in1=bo_sb[:, sl])
        nc.vector.tensor_add(out=u_sb, in0=u_sb, in1=dec_sb)

        def layer_norm(y_sb, g_ap, b_ap, name):
            """in-place LN on [1, D] row."""
            stats = small.tile([1, 2, 6], F32, name=f"st_{name}")
            for i in range(2):
                nc.vector.bn_stats(out=stats[:, i, :],
                                   in_=y_sb[:, i * 512:(i + 1) * 512])
            mv = small.tile([1, 2], F32, name=f"mv_{name}")
            nc.vector.bn_aggr(out=mv, in_=stats)
            rstd = small.tile([1, 1], F32, name=f"rs_{name}")
            nc.scalar.activation(rstd, mv[:, 1:2],
                                 mybir.ActivationFunctionType.Sqrt,
                                 bias=eps_sb, scale=1.0)
            nc.vector.reciprocal(rstd, rstd)
            nc.vector.tensor_scalar(out=y_sb, in0=y_sb,
                                    scalar1=mv[:, 0:1], scalar2=rstd,
                                    op0=mybir.AluOpType.subtract,
                                    op1=mybir.AluOpType.mult)
            nc.vector.tensor_mul(out=y_sb, in0=y_sb, in1=g_ap)
            nc.vector.tensor_add(out=y_sb, in0=y_sb, in1=b_ap)

        layer_norm(u_sb, g2_sb, l2_sb, "ln2")  # u_sb is now x

        # ---- FFN, tensor-parallel over F across cores ----
        # allgather x rows -> X_all [8, D] on every core
        bin_x = dram.tile([1, D], F32)
        nc.sync.dma_start(out=bin_x, in_=u_sb)
        bout_x = dram.tile([8, D], F32, addr_space="Shared")
        nc.gpsimd.collective_compute(
            "AllGather", mybir.AluOpType.bypass,
            replica_groups=[list(range(N_CORES))],
            ins=[bin_x], outs=[bout_x])
        Xall_sb = small.tile([8, D], F32, tag="rowtmp", bufs=2)
        nc.sync.dma_start(out=Xall_sb, in_=bout_x)
        # X_all^T in column layout [p, c, b]
        pxa = ps_sm.tile([P, NK, 8], F32, tag="bankA")
        for c in range(NK):
            nc.tensor.transpose(pxa[:, c, :], Xall_sb[:, c * P:(c + 1) * P],
                                ident[0:8, 0:8])
        XT_r = small.tile([P, NK, 8], F32R)
        nc.vector.tensor_copy(XT_r, pxa)
        # h1 slice = relu(X_all @ W1s + b1s)  [8, 512]
        ph8 = ps_sm.tile([8, 512], F32, tag="bankA")
        for c in range(NK):
            nc.tensor.matmul(ph8, XT_r[:, c, :], Wffn[:, c, 0:512],
                             start=(c == 0), stop=(c == NK - 1))
        h8 = small.tile([8, 512], F32, tag="rowtmp", bufs=2)
        nc.vector.tensor_add(out=h8, in0=ph8, in1=b1s_bc)
        nc.scalar.activation(h8, h8, mybir.ActivationFunctionType.Relu)
        pxb = ps_sm.tile([P, 4, 8], F32, tag="bankA")
        for c in range(4):
            nc.tensor.transpose(pxb[:, c, :], h8[:, c * P:(c + 1) * P],
                                ident[0:8, 0:8])
        hT_r = small.tile([P, 4, 8], F32R)
        nc.vector.tensor_copy(hT_r, pxb)
        # partial ff = h1s @ W2s  [8, D]
        pff = ps_ctx.tile([8, D], F32, tag="ctx")
        for nh in range(2):
            for c in range(4):
                nc.tensor.matmul(pff[:, nh * 512:(nh + 1) * 512],
                                 hT_r[:, c, :],
                                 Wffn[:, 2 * c + nh, 512:D],
                                 start=(c == 0), stop=(c == 3))
        ffp_sb = small.tile([8, D], F32, tag="rowtmp", bufs=2)
        nc.vector.tensor_copy(ffp_sb, pff)
        # reduce-scatter: core b receives row b of the summed ff
        bin_ff = dram.tile([8, D], F32)
        nc.sync.dma_start(out=bin_ff, in_=ffp_sb)
        bout_ff = dram.tile([1, D], F32)
        nc.gpsimd.collective_compute(
            "ReduceScatter", mybir.AluOpType.add,
            replica_groups=[list(range(N_CORES))],
            ins=[bin_ff], outs=[bout_ff])
        ff_row = small.tile([1, D], F32, tag="rowtmp", bufs=2)
        nc.sync.dma_start(out=ff_row, in_=bout_ff)
        # v = ff + b2 + x ; out = LN(v)
        v_sb = small.tile([1, D], F32)
        nc.vector.tensor_add(out=v_sb, in0=ff_row, in1=b2_sb)
        nc.vector.tensor_add(out=v_sb, in0=v_sb, in1=u_sb)
        layer_norm(v_sb, gf_sb, lf_sb, "lnf")

        nc.sync.dma_start(out=t["dout"], in_=v_sb)


def _in_maps(inputs):
    key = np.asarray(inputs["key"], np.float32)
    value = np.asarray(inputs["value"], np.float32)
    dec = np.asarray(inputs["decode_input"], np.float32)
    rep = {
        "Wq": np.asarray(inputs["Wq"], np.float32),
        "Wk": np.asarray(inputs["Wk"], np.float32),
        "Wv": np.asarray(inputs["Wv"], np.float32),
        "Wo": np.asarray(inputs["Wo"], np.float32),

        "bq": np.asarray(inputs["bq"], np.float32).reshape(1, D),
        "bv": np.asarray(inputs["bv"], np.float32).reshape(1, D),
        "bo": np.asarray(inputs["bo"], np.float32).reshape(1, D),

        "b2": np.asarray(inputs["b2"], np.float32).reshape(1, D),
        "ln2_g": np.asarray(inputs["ln2_g"], np.float32).reshape(1, D),
        "ln2_b": np.asarray(inputs["ln2_b"], np.float32).reshape(1, D),
        "lnf_g": np.asarray(inputs["lnf_g"], np.float32).reshape(1, D),
        "lnf_b": np.asarray(inputs["lnf_b"], np.float32).reshape(1, D),
    }
    W1 = np.asarray(inputs["W1"], np.float32)
    W2 = np.asarray(inputs["W2"], np.float32)
    b1 = np.asarray(inputs["b1"], np.float32)
    maps = []
    for b in range(N_CORES):
        m = dict(rep)
        m["key"] = np.ascontiguousarray(key[b])
        m["value"] = np.ascontiguousarray(value[b])
        m["dec"] = np.ascontiguousarray(dec[b].reshape(1, D))
        fs = slice(b * 512, (b + 1) * 512)
        m["W1s"] = np.ascontiguousarray(W1[:, fs])
        m["W2s"] = np.ascontiguousarray(W2[fs, :])
        m["b1s"] = np.ascontiguousarray(b1[fs].reshape(1, 512))
        maps.append(m)
    return maps


def get_runner():
    """Build (once) and return (nc, run_fn). run_fn(in_maps) -> per-core outs."""
    if "runner" in _CACHE:
        return _CACHE["runner"]
    nc = _build()
    from concourse.bass_utils import run_bass_kernel_spmd

    def run(in_maps):
        res = run_bass_kernel_spmd(nc, in_maps, core_ids=list(range(N_CORES)))
        return res.results

    _CACHE["runner"] = (nc, run)
    return _CACHE["runner"]


def kernel(**inputs):
    _, run = get_runner()
    results = run(_in_maps(inputs))
    out = np.stack([results[b]["out"] for b in range(N_CORES)], axis=0)
    return out.reshape(N_CORES, 1, D).astype(np.float32)

